# revision 1
# baseline (speedup 1.0000x reference)
"""Trainium2 Bass kernel for nn_LinearBase (a8w8 fp8 dynamic-quant GEMM).

y = fp8_quant_dyn(x) @ weight.T * x_scale * weight_scale.T + bias

Strategy (8 NeuronCores, SPMD):
  - Shard M (tokens) 8 ways: each core handles 512 rows of x and the full
    output columns. Weight / scales / bias replicated.
  - Host prep: weight codes (fp8e4m3fn grid, |w|<=448) are halved and cast
    to TRN fp8_e4m3 (|w|<=240 representable; halving keeps all normals
    exact), then transposed to [K, N] so the device streams [128,ksub,n]
    tiles directly.
  - Device: per 128-row tile of x: abs-max -> d = max(amax/224, 2e-12)
    (= 2*x_scale of the reference), r = 1/d, xq = fp8(x*r)  (= half of the
    reference's fp8 codes). PE-transpose xq into [K,M] layout. Then fp8
    DoubleRow matmuls (k=256 per MM) accumulate xq_half.T-free GEMM in
    PSUM = full_accum/4. Epilogue: ACT copy with scale=d (per-partition)
    then * (2*weight_scale) broadcast and + bias broadcast -> exact same
    fp32 rounding chain as the reference up to accumulation order.
"""

import sys

if "/opt/trn_rl_repo" not in sys.path:
    sys.path.insert(0, "/opt/trn_rl_repo")

import numpy as np
import ml_dtypes

N_CORES = 8
M_FULL, K_FULL, N_FULL = 4096, 4096, 16384
M_LOC = M_FULL // N_CORES  # 512

FP8 = ml_dtypes.float8_e4m3  # TRN FP8_EXP4 (IEEE-ish, max normal 240)


def build_nc(m_loc=M_LOC, k=K_FULL, n=N_FULL, n_tile=512, bc_chunk=2048,
             quant_mode="device", mm="f8", repeat=1, skip_epilogue=False):
    """quant_mode: 'device' (normal), 'device_nr' (NR-refined reciprocal),
    'host' (xq + d are kernel inputs; quant bypassed).
    mm: 'f8dr' (fp8 DoubleRow, k=256/MM), 'f8' (plain fp8, k=128/MM),
    'bf16' (bf16 operands, k=128/MM). Operands are always half-scaled codes."""
    import concourse.bass as bass  # noqa: F401
    import concourse.tile as tile
    from concourse import bacc, mybir
    from concourse.masks import make_identity

    f32 = mybir.dt.float32
    f8 = mybir.dt.float8e4
    mdt = mybir.dt.bfloat16 if mm == "bf16" else f8
    P = 128
    MT = m_loc // P        # m-tiles per core
    KB = k // P            # 128-row k blocks
    KT = k // 256          # DoubleRow k-tiles
    BC = bc_chunk
    NBC = n // BC          # broadcast chunks
    NPB = BC // n_tile     # n-tiles per broadcast chunk
    assert m_loc % P == 0 and k % 256 == 0 and n % BC == 0 and BC % n_tile == 0

    nc = bacc.Bacc("TRN2", target_bir_lowering=False, debug=False)
    if quant_mode == "host":
        xq_d = nc.dram_tensor("xq", [m_loc, k], mdt, kind="ExternalInput")
        d_d = nc.dram_tensor("d", [m_loc, 1], f32, kind="ExternalInput")
    elif quant_mode == "host_t":
        xqt_d = nc.dram_tensor("xqT", [k, m_loc], mdt, kind="ExternalInput")
        d_d = nc.dram_tensor("d", [m_loc, 1], f32, kind="ExternalInput")
    else:
        x_d = nc.dram_tensor("x", [m_loc, k], f32, kind="ExternalInput")
    w_d = nc.dram_tensor("wT", [k, n], mdt, kind="ExternalInput")
    ws_d = nc.dram_tensor("ws2", [n], f32, kind="ExternalInput")
    b_d = nc.dram_tensor("bias", [n], f32, kind="ExternalInput")
    y_d = nc.dram_tensor("y", [m_loc, n], f32, kind="ExternalOutput")

    w_v = w_d.ap().rearrange("(ko p) n -> p ko n", p=P)  # [128, KB, n]

    DR = mybir.MatmulPerfMode.DoubleRow
    Copy = mybir.ActivationFunctionType.Copy

    with tile.TileContext(nc) as tc:
        with (
            tc.tile_pool(name="const", bufs=1) as const_pool,
            tc.tile_pool(name="xin", bufs=2) as x_pool,
            tc.tile_pool(name="xq", bufs=2) as xq_pool,
            tc.tile_pool(name="xqt", bufs=1) as xqt_pool,
            tc.tile_pool(name="wp", bufs=(2 if mm == "bf16" else 4)) as w_pool,
            tc.tile_pool(name="bcast", bufs=2) as bc_pool,
            tc.tile_pool(name="outp", bufs=4) as out_pool,
            tc.tile_pool(name="sc", bufs=1) as sc_pool,
            tc.tile_pool(name="tmp", bufs=2) as tmp_pool,
            tc.tile_pool(name="tps", bufs=2, space="PSUM") as tpsum,
            tc.tile_pool(name="mps", bufs=6, space="PSUM") as mpsum,
        ):
            ident = const_pool.tile([P, P], mdt)
            make_identity(nc, ident)

            d_all = sc_pool.tile([P, MT], f32)
            r_all = sc_pool.tile([P, MT], f32)
            xqt_mi = []
            for _i in range(MT):
                xqt_tile = xqt_pool.tile([P, KB, P], mdt, tag=f"xqt{_i}")
                xqt_mi.append(xqt_tile)

            for _rep in range(repeat):
                # ---- Phase A: dynamic per-token quantization + transpose ----
                for mi in range(MT):
                    if quant_mode == "host_t":
                        xqt_v = xqt_d.ap().rearrange("(ko p) m -> p ko m", p=P)
                        nc.sync.dma_start(xqt_mi[mi][:], xqt_v[:, :, mi * P:(mi + 1) * P])
                        nc.sync.dma_start(d_all[:, mi:mi + 1], d_d[mi * P:(mi + 1) * P, :])
                        continue
                    if quant_mode == "host":
                        xqm = xq_pool.tile([P, k], mdt)
                        nc.sync.dma_start(xqm[:], xq_d[mi * P:(mi + 1) * P, :])
                        nc.sync.dma_start(d_all[:, mi:mi + 1], d_d[mi * P:(mi + 1) * P, :])
                    else:
                        xt = x_pool.tile([P, k], f32)
                        nc.sync.dma_start(xt[:], x_d[mi * P:(mi + 1) * P, :])
                        amax = tmp_pool.tile([P, 1], f32)
                        nc.vector.tensor_reduce(
                            amax[:], xt[:],
                            axis=mybir.AxisListType.X,
                            op=mybir.AluOpType.max,
                            apply_absolute_value=True,
                        )
                        # d = max(amax/224, 2e-12) == 2 * reference x_scale
                        nc.vector.tensor_scalar(
                            out=d_all[:, mi:mi + 1], in0=amax[:],
                            scalar1=float(np.float32(1.0) / np.float32(224.0)),
                            scalar2=2e-12,
                            op0=mybir.AluOpType.mult,
                            op1=mybir.AluOpType.max,
                        )
                        if quant_mode != "device_div":
                            nc.vector.reciprocal(r_all[:, mi:mi + 1], d_all[:, mi:mi + 1])
                        if quant_mode == "device_nr":
                            # Newton-Raphson refine: r = r*(2 - d*r)
                            t1 = tmp_pool.tile([P, 1], f32, tag="nr1")
                            nc.vector.tensor_mul(t1[:], r_all[:, mi:mi + 1], d_all[:, mi:mi + 1])
                            nc.vector.tensor_scalar(
                                out=t1[:], in0=t1[:], scalar1=-1.0, scalar2=2.0,
                                op0=mybir.AluOpType.mult, op1=mybir.AluOpType.add,
                            )
                            nc.vector.tensor_mul(r_all[:, mi:mi + 1], r_all[:, mi:mi + 1], t1[:])
                        xq8 = xq_pool.tile([P, k], f8, tag="xq8")
                        if quant_mode in ("device_vmul", "device_nr"):
                            nc.vector.tensor_scalar(
                                out=xq8[:], in0=xt[:], scalar1=r_all[:, mi:mi + 1],
                                scalar2=None, op0=mybir.AluOpType.mult,
                            )
                        else:
                            nc.scalar.activation(xq8[:], xt[:], Copy, scale=r_all[:, mi:mi + 1])
                        if mm == "bf16":
                            xqm = xq_pool.tile([P, k], mdt, tag="xqb")
                            nc.vector.tensor_copy(xqm[:], xq8[:])
                        else:
                            xqm = xq8
                    for kb in range(KB):
                        if mm == "bf16":
                            tp = tpsum.tile([P, P], mdt, tag="tpb")
                            nc.tensor.transpose(tp[:], xqm[:, kb * P:(kb + 1) * P], ident[:])
                            nc.vector.tensor_copy(xqt_mi[mi][:, kb, :], tp[:])
                        else:
                            # fp8 PE-transpose requires output element step of 2 in PSUM
                            tp = tpsum.tile([P, P, 2], f8, tag="tp8")
                            nc.tensor.transpose(tp[:, :, 0], xqm[:, kb * P:(kb + 1) * P], ident[:])
                            nc.vector.tensor_copy(xqt_mi[mi][:, kb, :], tp[:, :, 0])

                # ---- Phase B: DoubleRow GEMM + fused dequant epilogue ----
                for nb in range(NBC):
                    wsb = bc_pool.tile([P, BC], f32, tag="wsb")
                    bb = bc_pool.tile([P, BC], f32, tag="bb")
                    nc.sync.dma_start(wsb[:], ws_d[nb * BC:(nb + 1) * BC][None].to_broadcast((P, BC)))
                    nc.sync.dma_start(bb[:], b_d[nb * BC:(nb + 1) * BC][None].to_broadcast((P, BC)))
                    for nj in range(NPB):
                        ni = nb * NPB + nj
                        wt = w_pool.tile([P, KB, n_tile], mdt)
                        nc.sync.dma_start(wt[:], w_v[:, :, ni * n_tile:(ni + 1) * n_tile])
                        for mi in range(MT):
                            ps = mpsum.tile([P, n_tile], f32)
                            if mm == "f8dr":
                                for j in range(KT):
                                    nc.tensor.matmul(
                                        ps[:],
                                        xqt_mi[mi][:, 2 * j:2 * j + 2, :],
                                        wt[:, 2 * j:2 * j + 2, :],
                                        start=(j == 0),
                                        stop=(j == KT - 1),
                                        perf_mode=DR,
                                    )
                            else:
                                for j in range(KB):
                                    nc.tensor.matmul(
                                        ps[:],
                                        xqt_mi[mi][:, j, :],
                                        wt[:, j, :],
                                        start=(j == 0),
                                        stop=(j == KB - 1),
                                    )
                            ot = out_pool.tile([P, n_tile], f32)
                            nc.scalar.activation(ot[:], ps[:], Copy, scale=d_all[:, mi:mi + 1])
                            if not skip_epilogue:
                                nc.vector.tensor_mul(ot[:], ot[:], wsb[:, nj * n_tile:(nj + 1) * n_tile])
                                nc.vector.tensor_add(ot[:], ot[:], bb[:, nj * n_tile:(nj + 1) * n_tile])
                            nc.sync.dma_start(
                                y_d[mi * P:(mi + 1) * P, ni * n_tile:(ni + 1) * n_tile], ot[:]
                            )
    nc.finalize()
    return nc


_CACHE = {}


def host_prep(weight, weight_scale, bias):
    wT = np.ascontiguousarray((weight.astype(np.float32) * np.float32(0.5)).astype(FP8).T)
    ws2 = np.ascontiguousarray((weight_scale.astype(np.float32).reshape(-1) * np.float32(2.0)))
    b = np.ascontiguousarray(bias.astype(np.float32).reshape(-1))
    return wT, ws2, b


def _stub_axon_hooks():
    # this axon client ships no antenv.axon_hooks; make trace requests
    # degrade to untraced runs instead of crashing on import
    import types
    if "antenv.axon_hooks" not in sys.modules:
        m = types.ModuleType("antenv.axon_hooks")
        m.get_axon_ntff_profile_hook = lambda: None
        sys.modules["antenv.axon_hooks"] = m


def kernel(x, weight, weight_scale, bias):
    _stub_axon_hooks()
    from concourse.bass_utils import run_bass_kernel_spmd

    if "nc" not in _CACHE:
        _CACHE["nc"] = build_nc()
    nc = _CACHE["nc"]

    x = np.ascontiguousarray(x.astype(np.float32))
    wT, ws2, b = host_prep(weight, weight_scale, bias)

    in_maps = [
        {"x": x[i * M_LOC:(i + 1) * M_LOC], "wT": wT, "ws2": ws2, "bias": b}
        for i in range(N_CORES)
    ]
    res = run_bass_kernel_spmd(nc, in_maps, list(range(N_CORES)))
    _CACHE["last_results"] = res
    y = np.concatenate([res.results[i]["y"] for i in range(N_CORES)], axis=0)
    return y



# revision 5
# speedup vs baseline: 12.8137x; 12.8137x over previous
"""Trainium2 Bass kernel for nn_LinearBase (a8w8 fp8 dynamic-quant GEMM).

y = fp8_quant_dyn(x) @ weight.T * x_scale * weight_scale.T + bias
M, K, N = 4096, 4096, 16384 split over 8 cores as 4 M-shards x 2 N-shards.

Per-core (M_LOC=1024, N_LOC=8192):
  - Host prep: weight codes (fp8e4m3fn grid, |w|<=448) are halved and cast
    to TRN fp8_e4m3 (halving keeps all normals exact), then laid out as
    [ni, p, ko, nj] so each 512-column chunk streams as one fully
    contiguous DMA. ws2 = 2*weight_scale and bias are passed in bf16.
  - Device phase A (per 128-token tile): abs-max -> d = max(amax/224,
    2e-12) (= 2*x_scale of the reference), r = 1/d, xq = fp8(x*r) (= half
    the reference codes). PE-transpose xq into [K, M] layout.
  - Device phase B: fp8 DoubleRow matmuls (k=256/instr) accumulate
    PSUM = full_accum/4; epilogue on DVE: yt = ((psum * d) * ws2) + bias
    (scalar_tensor_tensor + tensor_add), emitted in bf16; host upcasts.
  - DMA queues: weight stream on SP, x loads + y stores on Activation, so
    a stalled store never blocks the weight pipeline.
  - With repeat>1 (timing builds), the next rep's phase A is interleaved
    into this rep's chunk loop so the PE never drains at rep boundaries.
"""

import sys

if "/opt/trn_rl_repo" not in sys.path:
    sys.path.insert(0, "/opt/trn_rl_repo")

import numpy as np
import ml_dtypes

N_CORES = 8
M_FULL, K_FULL, N_FULL = 4096, 4096, 16384
MS, NS = 4, 2                # M-shards x N-shards
M_LOC = M_FULL // MS         # 1024
N_LOC = N_FULL // NS         # 8192

FP8 = ml_dtypes.float8_e4m3  # TRN FP8_EXP4 (max normal 240)
BF16 = ml_dtypes.bfloat16


def build_nc(m_loc=M_LOC, k=K_FULL, n_loc=N_LOC, n_tile=512, repeat=1,
             w_bufs=2, y_dt="bf16", interleave=True):
    import concourse.bass as bass  # noqa: F401
    import concourse.tile as tile
    from concourse import bacc, mybir
    from concourse.masks import make_identity

    f32 = mybir.dt.float32
    f8 = mybir.dt.float8e4
    bf16 = mybir.dt.bfloat16
    ydt = bf16 if y_dt == "bf16" else f32
    P = 128
    MT = m_loc // P              # 8 token tiles
    KB = k // P                  # 32 k blocks
    KT = k // 256                # 16 DoubleRow k-tiles
    NT = n_loc // n_tile         # 16 n chunks
    assert m_loc % P == 0 and k % 256 == 0 and n_loc % n_tile == 0
    DR = mybir.MatmulPerfMode.DoubleRow
    Copy = mybir.ActivationFunctionType.Copy
    mult = mybir.AluOpType.mult

    nc = bacc.Bacc("TRN2", target_bir_lowering=False, debug=False)
    x_d = nc.dram_tensor("x", [m_loc, k], f32, kind="ExternalInput")
    # host layout: row = ni*128 + p, col = ko*n_tile + nj
    w_d = nc.dram_tensor("w5", [NT * P, KB * n_tile], f8, kind="ExternalInput")
    ws_d = nc.dram_tensor("ws2", [n_loc], bf16, kind="ExternalInput")
    b_d = nc.dram_tensor("bias", [n_loc], bf16, kind="ExternalInput")
    y_d = nc.dram_tensor("y", [m_loc, n_loc], ydt, kind="ExternalOutput")

    can_interleave = interleave and NT >= 2 * MT

    with tile.TileContext(nc) as tc:
        with (
            tc.tile_pool(name="const", bufs=1) as const_pool,
            tc.tile_pool(name="bcast", bufs=1) as bc_pool,
            tc.tile_pool(name="xin", bufs=2) as x_pool,
            tc.tile_pool(name="xq", bufs=2) as xq_pool,
            tc.tile_pool(name="xqt", bufs=2) as xqt_pool,
            tc.tile_pool(name="wp", bufs=w_bufs) as w_pool,
            tc.tile_pool(name="yt", bufs=4) as yt_pool,
            tc.tile_pool(name="sc", bufs=2) as sc_pool,
            tc.tile_pool(name="tmp", bufs=2) as tmp_pool,
            tc.tile_pool(name="tps", bufs=2, space="PSUM") as tpsum,
            tc.tile_pool(name="mps", bufs=6, space="PSUM") as mpsum,
        ):
            # --- weight prefetch state (SP queue) ---
            w_tiles = {}

            def wload(rep, ni):
                t = w_pool.tile([P, KB, n_tile], f8, tag="wt")
                src = w_d[ni * P:(ni + 1) * P, :].rearrange(
                    "p (ko n) -> p ko n", n=n_tile)
                nc.sync.dma_start(t[:], src)
                w_tiles[(rep, ni)] = t

            wload(0, 0)
            if NT > 1:
                wload(0, 1)

            ident = const_pool.tile([P, P], f8)
            make_identity(nc, ident)

            wsb = bc_pool.tile([P, n_loc], bf16, tag="wsb")
            bb = bc_pool.tile([P, n_loc], bf16, tag="bb")
            nc.sync.dma_start(wsb[:], ws_d[None].to_broadcast((P, n_loc)))
            nc.sync.dma_start(bb[:], b_d[None].to_broadcast((P, n_loc)))

            # --- per-rep phase A state ---
            class Rep:
                __slots__ = ("d", "r", "xt", "xq8", "xqt")

            def new_state():
                st = Rep()
                st.d = sc_pool.tile([P, MT], f32, tag="d")
                st.r = sc_pool.tile([P, MT], f32, tag="r")
                st.xt = [None] * MT
                st.xq8 = [None] * MT
                st.xqt = []
                for i in range(MT):
                    xqt_tile = xqt_pool.tile([P, KB, P], f8, tag=f"xqt{i}")
                    st.xqt.append(xqt_tile)
                return st

            def xload(st, mi):
                xt = x_pool.tile([P, k], f32, tag="xt")
                nc.scalar.dma_start(xt[:], x_d[mi * P:(mi + 1) * P, :])
                st.xt[mi] = xt

            def quant(st, mi):
                xt = st.xt[mi]
                amax = tmp_pool.tile([P, 1], f32, tag="amax")
                nc.vector.tensor_reduce(
                    amax[:], xt[:],
                    axis=mybir.AxisListType.X,
                    op=mybir.AluOpType.max,
                    apply_absolute_value=True,
                )
                # d = max(amax/224, 2e-12) == 2 * reference x_scale
                nc.vector.tensor_scalar(
                    out=st.d[:, mi:mi + 1], in0=amax[:],
                    scalar1=float(np.float32(1.0) / np.float32(224.0)),
                    scalar2=2e-12,
                    op0=mybir.AluOpType.mult,
                    op1=mybir.AluOpType.max,
                )
                nc.vector.reciprocal(st.r[:, mi:mi + 1], st.d[:, mi:mi + 1])
                xq8 = xq_pool.tile([P, k], f8, tag="xq8")
                nc.scalar.activation(xq8[:], xt[:], Copy, scale=st.r[:, mi:mi + 1])
                st.xq8[mi] = xq8

            def tpose(st, mi, kb0, kb1):
                xq8 = st.xq8[mi]
                for kb in range(kb0, kb1):
                    # fp8 PE-transpose requires output element step of 2 in PSUM
                    tp = tpsum.tile([P, P, 2], f8, tag="tp")
                    nc.tensor.transpose(tp[:, :, 0], xq8[:, kb * P:(kb + 1) * P], ident[:])
                    nc.scalar.activation(st.xqt[mi][:, kb, :], tp[:, :, 0], Copy)

            def phase_a_full(st):
                xload(st, 0)
                if MT > 1:
                    xload(st, 1)
                for mi in range(MT):
                    quant(st, mi)
                    if mi + 2 < MT:
                        xload(st, mi + 2)
                    tpose(st, mi, 0, KB)

            def frag(st, ni):
                # phase A of the NEXT rep, spread across this rep's chunks.
                # quant(m) lands a full chunk before its transposes so the
                # PE's inter-chunk transpose batch never waits on the ACT.
                if ni == 0:
                    xload(st, 0)
                    xload(st, 1)
                elif ni % 2 == 1:
                    m = (ni - 1) // 2
                    quant(st, m)
                    if m + 2 < MT:
                        xload(st, m + 2)
                    if m >= 1:
                        tpose(st, m - 1, KB // 2, KB)
                else:
                    m = (ni - 2) // 2
                    tpose(st, m, 0, KB // 2)

            def phase_b(rep, cur, nxt):
                for ni in range(NT):
                    if nxt is not None and can_interleave:
                        frag(nxt, ni)
                    # prefetch weights two chunks ahead (possibly next rep)
                    gl = ni + 2
                    if gl < NT:
                        wload(rep, gl)
                    elif rep + 1 < repeat:
                        wload(rep + 1, gl - NT)
                    wt = w_tiles.pop((rep, ni))
                    wv = wt[:]
                    sl = slice(ni * n_tile, (ni + 1) * n_tile)
                    for mi in range(MT):
                        ps = mpsum.tile([P, n_tile], f32)
                        for j in range(KT):
                            nc.tensor.matmul(
                                ps[:],
                                cur.xqt[mi][:, 2 * j:2 * j + 2, :],
                                wv[:, 2 * j:2 * j + 2, :],
                                start=(j == 0),
                                stop=(j == KT - 1),
                                perf_mode=DR,
                            )
                        yt = yt_pool.tile([P, n_tile], ydt)
                        # yt = (psum * d) * ws2 ; then yt += bias
                        nc.vector.scalar_tensor_tensor(
                            yt[:], ps[:], cur.d[:, mi:mi + 1], wsb[:, sl],
                            mult, mult,
                        )
                        nc.vector.tensor_add(yt[:], yt[:], bb[:, sl])
                        nc.sync.dma_start(
                            y_d[mi * P:(mi + 1) * P, sl], yt[:]
                        )
                if nxt is not None and can_interleave:
                    tpose(nxt, MT - 1, 0, KB)  # tail fragment

            cur = new_state()
            phase_a_full(cur)
            for rep in range(repeat):
                if rep + 1 < repeat:
                    nxt = new_state()
                else:
                    nxt = None
                phase_b(rep, cur, nxt)
                if nxt is not None and not can_interleave:
                    phase_a_full(nxt)
                cur = nxt
    nc.finalize()
    return nc


_CACHE = {}


def host_prep(weight, weight_scale, bias):
    P, KB, NT, n_tile = 128, K_FULL // 128, N_LOC // 512, 512
    W8 = (weight.astype(np.float32) * np.float32(0.5)).astype(FP8)  # [N, K]
    w5s, wss, bbs = [], [], []
    ws_flat = weight_scale.astype(np.float32).reshape(-1) * np.float32(2.0)
    b_flat = bias.astype(np.float32).reshape(-1)
    for s in range(NS):
        Ws = W8[s * N_LOC:(s + 1) * N_LOC]          # [N_LOC, K]
        wt4 = Ws.T.reshape(KB, P, NT, n_tile)       # [ko, p, ni, nj]
        w5 = np.ascontiguousarray(
            wt4.transpose(2, 1, 0, 3).reshape(NT * P, KB * n_tile))
        w5s.append(w5)
        wss.append(ws_flat[s * N_LOC:(s + 1) * N_LOC].astype(BF16))
        bbs.append(b_flat[s * N_LOC:(s + 1) * N_LOC].astype(BF16))
    return w5s, wss, bbs


def per_core_inputs(x, w5s, wss, bbs):
    maps = []
    for c in range(N_CORES):
        ms, ns = c // NS, c % NS
        maps.append({
            "x": np.ascontiguousarray(x[ms * M_LOC:(ms + 1) * M_LOC]),
            "w5": w5s[ns],
            "ws2": wss[ns],
            "bias": bbs[ns],
        })
    return maps


def _stub_axon_hooks():
    # this axon client ships no antenv.axon_hooks; make trace requests
    # degrade to untraced runs instead of crashing on import
    import types
    if "antenv.axon_hooks" not in sys.modules:
        m = types.ModuleType("antenv.axon_hooks")
        m.get_axon_ntff_profile_hook = lambda: None
        sys.modules["antenv.axon_hooks"] = m


def kernel(x, weight, weight_scale, bias):
    _stub_axon_hooks()
    from concourse.bass_utils import run_bass_kernel_spmd

    if "nc" not in _CACHE:
        _CACHE["nc"] = build_nc()
    nc = _CACHE["nc"]

    x = np.ascontiguousarray(x.astype(np.float32))
    w5s, wss, bbs = host_prep(weight, weight_scale, bias)
    in_maps = per_core_inputs(x, w5s, wss, bbs)
    res = run_bass_kernel_spmd(nc, in_maps, list(range(N_CORES)))
    _CACHE["last_results"] = res
    y = np.empty((M_FULL, N_FULL), np.float32)
    for c in range(N_CORES):
        ms, ns = c // NS, c % NS
        y[ms * M_LOC:(ms + 1) * M_LOC, ns * N_LOC:(ns + 1) * N_LOC] = (
            res.results[c]["y"].astype(np.float32))
    return y
